# revision 6
# baseline (speedup 1.0000x reference)
"""Haar 3D wavelet transform (2x2x2 stride-2 conv, 8 sign filters) on 8 trn2 cores.

Input  x: (2, 3, 33, 512, 512) f32, w: (8, 1, 2, 2, 2) f32.
Output:   (2, 24, 17, 256, 256) f32.

Memory-bound problem -> move bytes as bf16 (tolerance 2e-2, bf16 round-trip
costs ~4e-3).  The host packs each (b, c, t_out) unit's two input frames
(x[2t-1], x[2t]; frame 0 replicated for t=0) so that the 8 taps of every
2x2x2 block land in 8 different SBUF partitions:

    partition p = dt*64 + dh*32 + dw*16 + g      (g = row-group 0..15)
    free      f = j*256 + wp                      (row ho = g*16 + j, col wp)

Then ONE stationary 128x128 matrix computes all 8 filter outputs per block:

    psum[k*16 + g, f] = sum_{dt,dh,dw} w[k,0,dt,dh,dw] * x[(dt,dh,dw,g), f]

i.e. the whole transform is a per-column 128x128 matmul.  All 102*4096
columns are independent, so they are split exactly 52224 per core (no
padding waste).  On-chip per tile: DMA-in (sync) -> 6 matmuls (PE, bf16)
-> PSUM->SBUF cast copies (split scalar/vector) -> DMA-out (scalar).
"""

import numpy as np

N_CORES = 8
B, C, T_IN, H, W = 2, 3, 33, 512, 512
T_OUT, HO, WO = 17, 256, 256
N_UNITS = B * C * T_OUT                      # 102
UNIT_COLS = 4096                             # free columns per unit
COLS_TOTAL = N_UNITS * UNIT_COLS             # 417792
COLS_PER_CORE = COLS_TOTAL // N_CORES        # 52224
F_TILE = 3072                                # 6 x 512-wide matmul chunks
N_TILES = COLS_PER_CORE // F_TILE            # 17


def _build_nc(legalize=True):
    import concourse.bass as bass
    import concourse.mybir as mybir
    from concourse.tile import TileContext

    nc = bass.Bass()
    xin = nc.declare_dram_parameter(
        "xin", [N_TILES, 128, F_TILE], mybir.dt.bfloat16, isOutput=False)
    wmat = nc.declare_dram_parameter(
        "wmat", [128, 128], mybir.dt.bfloat16, isOutput=False)
    yout = nc.declare_dram_parameter(
        "yout", [N_TILES, 128, F_TILE], mybir.dt.bfloat16, isOutput=True)

    with TileContext(nc) as tc:
        with (
            tc.tile_pool(name="const", bufs=1) as cpool,
            tc.tile_pool(name="xpool", bufs=5) as xpool,
            tc.tile_pool(name="ypool", bufs=5) as ypool,
            tc.tile_pool(name="ppool", bufs=8, space="PSUM") as ppool,
        ):
            wt = cpool.tile([128, 128], mybir.dt.bfloat16)
            nc.sync.dma_start(out=wt[:], in_=wmat[:])

            for i in range(N_TILES):
                xt = xpool.tile([128, F_TILE], mybir.dt.bfloat16)
                nc.sync.dma_start(out=xt[:], in_=xin[i])

                yt = ypool.tile([128, F_TILE], mybir.dt.bfloat16)
                for m in range(F_TILE // 512):
                    f0 = m * 512
                    pt = ppool.tile([128, 512], mybir.dt.float32)
                    nc.tensor.matmul(
                        pt[:], lhsT=wt[:], rhs=xt[:, f0:f0 + 512],
                        start=True, stop=True)
                    if m % 2 == 0:
                        nc.scalar.copy(yt[:, f0:f0 + 512], pt[:])
                    else:
                        nc.vector.tensor_copy(yt[:, f0:f0 + 512], pt[:])

                nc.scalar.dma_start(out=yout[i], in_=yt[:])

    if legalize:
        _legalize_waits(nc)
    return nc


def _legalize_waits(nc, limit=1):
    """walrus codegen rejects instructions carrying more than ~1 sem wait
    (e.g. Matmult's LoadWeights slot).  Move excess waits onto NoOp
    instructions inserted just before the instruction on the same engine
    queue -- semantically identical (all waits still precede execution)."""
    import bass_rust

    fn = nc.m.functions[0]
    lastblk = fn.blocks[-1]
    eng_ns = {
        "PE": nc.tensor, "DVE": nc.vector, "Activation": nc.scalar,
        "SP": nc.sync, "Pool": nc.gpsimd,
    }
    # NoOp codegen requires >=1 sem update. Give each engine its own dummy
    # sem (ids picked from the top of the 150..255 HW range, skipping any id
    # already referenced) so no counting or cross-proc rule is disturbed.
    used_ids = set()
    for blk in fn.blocks:
        for inst in blk.instructions:
            si = getattr(inst, "sync_info", None)
            if si is None:
                continue
            for w in si.on_wait:
                used_ids.add(w.id)
            for upd in si.on_update:
                used_ids.add(upd.id)
    avail = [i for i in range(255, 149, -1) if i not in used_ids]
    eng_upd = {}
    for k, en in enumerate(["PE", "DVE", "Activation", "SP", "Pool"]):
        eng_upd[en] = bass_rust.SyncUpdate(
            sync_type="semaphore", id=avail[k], ant_name=f"waitnop_{en}",
            update_mode="sem-inc", update_value=1, update_reg=None)

    def copy_wait(w):
        return bass_rust.SyncWait(
            sync_type=w.sync_type, id=w.id, ant_name=w.ant_name,
            wait_mode=w.wait_mode, wait_value=w.wait_value, wait_reg=w.wait_reg)

    def make_nop(engine_name, waits):
        ns = eng_ns[engine_name]
        ns.nop(hint="waitcarrier")
        nop = lastblk.instructions.pop()
        raw = getattr(nop, "inst", nop)
        raw.sync_info = bass_rust.SyncInfo(
            on_wait=[copy_wait(w) for w in waits],
            on_update=[eng_upd[engine_name]])
        return raw

    for blk in fn.blocks:
        insts = blk.instructions
        i = 0
        while i < len(insts):
            inst = insts[i]
            ty = type(inst).__name__
            si = getattr(inst, "sync_info", None)
            if (ty not in ("InstEventSemaphore", "InstNoOp")
                    and si is not None and len(si.on_wait) > limit):
                ename = str(inst.engine).split(".")[-1]
                waits = [copy_wait(w) for w in si.on_wait]
                upds = list(si.on_update)
                extra, keep = waits[:-limit], waits[-limit:]
                for w in extra:
                    insts.insert(i, make_nop(ename, [w]))
                    i += 1
                inst.sync_info = bass_rust.SyncInfo(
                    on_wait=keep, on_update=upds)
            i += 1


def _make_wmat(w):
    """128x128 stationary butterfly: wm[p, q] with p = dt*64+dh*32+dw*16+g,
    q = k*16+g, value w[k,0,dt,dh,dw].  Fully general in w."""
    w = np.asarray(w, dtype=np.float32).reshape(8, 2, 2, 2)
    wm = np.zeros((128, 128), dtype=np.float32)
    g = np.arange(16)
    for k in range(8):
        for dt in range(2):
            for dh in range(2):
                for dw in range(2):
                    wm[dt * 64 + dh * 32 + dw * 16 + g, k * 16 + g] = \
                        w[k, dt, dh, dw]
    return wm


def _pack_input(x16):
    """(B,C,T_IN,512,512) bf16 -> (128, COLS_TOTAL) device column layout."""
    t = np.arange(T_OUT)
    t0 = np.maximum(2 * t - 1, 0)
    t1 = 2 * t
    fp = np.stack([x16[:, :, t0], x16[:, :, t1]], axis=3)  # b c t dt 512 512
    v = fp.reshape(N_UNITS, 2, 16, 16, 2, 256, 2)          # u dt g j dh wp dw
    v = v.transpose(0, 1, 4, 6, 2, 3, 5)                   # u dt dh dw g j wp
    p = v.reshape(N_UNITS, 128, UNIT_COLS)
    return p.transpose(1, 0, 2).reshape(128, COLS_TOTAL)


def _unpack_output(yg):
    """(128, COLS_TOTAL) bf16 device layout -> (2, 24, 17, 256, 256) f32."""
    q = yg.reshape(128, N_UNITS, UNIT_COLS).transpose(1, 0, 2)
    planes = q.reshape(N_UNITS, 8, HO, WO)                 # u k (g j)=ho wp
    out = planes.reshape(B, C, T_OUT, 8, HO, WO)
    return np.ascontiguousarray(
        out.transpose(0, 3, 1, 2, 4, 5)).reshape(
        B, 8 * C, T_OUT, HO, WO).astype(np.float32)


LAST_RESULT = None


def kernel(x, w):
    import os
    import ml_dtypes
    from concourse.bass_utils import run_bass_kernel_spmd

    bf16 = ml_dtypes.bfloat16
    x16 = np.asarray(x, dtype=np.float32).astype(bf16)
    wm = _make_wmat(w).astype(bf16)

    g = _pack_input(x16)
    in_maps = []
    for m in range(N_CORES):
        sl = g[:, m * COLS_PER_CORE:(m + 1) * COLS_PER_CORE]
        t3 = np.ascontiguousarray(
            sl.reshape(128, N_TILES, F_TILE).transpose(1, 0, 2))
        in_maps.append({"xin": t3, "wmat": wm})

    nc = _build_nc()
    kw = {}
    if os.environ.get("KERNEL_PROFILE") == "1":
        kw = dict(trace=True, tmpdir=os.environ.get("KERNEL_PROFILE_DIR"))
    res = run_bass_kernel_spmd(nc, in_maps, core_ids=list(range(N_CORES)), **kw)
    global LAST_RESULT
    LAST_RESULT = res

    yg = np.concatenate(
        [np.asarray(res.results[m]["yout"]).transpose(1, 0, 2).reshape(
            128, COLS_PER_CORE) for m in range(N_CORES)], axis=1)
    return _unpack_output(yg)


if __name__ == "__main__":
    x = np.random.randn(B, C, T_IN, H, W).astype(np.float32)
    SCALE = 0.3536
    flags = np.array([[0, 0, 0], [0, 0, 1], [0, 1, 0], [0, 1, 1],
                      [1, 0, 0], [1, 0, 1], [1, 1, 0], [1, 1, 1]])
    t, h, ww = np.meshgrid(np.arange(2), np.arange(2), np.arange(2), indexing="ij")
    sign = (-1.0) ** (flags[:, 0, None, None, None] * t
                      + flags[:, 1, None, None, None] * h
                      + flags[:, 2, None, None, None] * ww)
    wf = (SCALE * sign).reshape(8, 1, 2, 2, 2).astype(np.float32)
    y = kernel(x, wf)
    print(y.shape, y.dtype)


# revision 8
# speedup vs baseline: 1.0898x; 1.0898x over previous
"""Haar 3D wavelet transform (2x2x2 stride-2 conv, 8 sign filters) on 8 trn2 cores.

Input  x: (2, 3, 33, 512, 512) f32, w: (8, 1, 2, 2, 2) f32.
Output:   (2, 24, 17, 256, 256) f32.

Memory-bound problem -> move bytes as bf16 (tolerance 2e-2, bf16 round-trip
costs ~4e-3).  The host packs each (b, c, t_out) unit's two input frames
(x[2t-1], x[2t]; frame 0 replicated for t=0) so that the 8 taps of every
2x2x2 block land in 8 different SBUF partitions:

    partition p = dt*64 + dh*32 + dw*16 + g      (g = row-group 0..15)
    free      f = j*256 + wp                      (row ho = g*16 + j, col wp)

Then ONE stationary 128x128 matrix computes all 8 filter outputs per block:

    psum[k*16 + g, f] = sum_{dt,dh,dw} w[k,0,dt,dh,dw] * x[(dt,dh,dw,g), f]

i.e. the whole transform is a per-column 128x128 matmul.  All 102*4096
columns are independent, so they are split exactly 52224 per core (no
padding waste).  On-chip per tile: DMA-in (sync) -> 6 matmuls (PE, bf16)
-> PSUM->SBUF cast copies (split scalar/vector) -> DMA-out (scalar).
"""

import numpy as np

N_CORES = 8
B, C, T_IN, H, W = 2, 3, 33, 512, 512
T_OUT, HO, WO = 17, 256, 256
N_UNITS = B * C * T_OUT                      # 102
UNIT_COLS = 4096                             # free columns per unit
COLS_TOTAL = N_UNITS * UNIT_COLS             # 417792
COLS_PER_CORE = COLS_TOTAL // N_CORES        # 52224
F_TILE = 3072                                # 6 x 512-wide matmul chunks
N_TILES = COLS_PER_CORE // F_TILE            # 17


def _build_nc(legalize=True):
    import concourse.bass as bass
    import concourse.mybir as mybir
    from concourse.tile import TileContext

    nc = bass.Bass()
    xin = nc.declare_dram_parameter(
        "xin", [N_TILES, 128, F_TILE], mybir.dt.bfloat16, isOutput=False)
    wmat = nc.declare_dram_parameter(
        "wmat", [128, 128], mybir.dt.bfloat16, isOutput=False)
    yout = nc.declare_dram_parameter(
        "yout", [N_TILES, 128, F_TILE], mybir.dt.bfloat16, isOutput=True)

    with TileContext(nc) as tc:
        with (
            tc.tile_pool(name="const", bufs=1) as cpool,
            tc.tile_pool(name="xpool", bufs=8) as xpool,
            tc.tile_pool(name="ypool", bufs=6) as ypool,
            tc.tile_pool(name="ppool", bufs=8, space="PSUM") as ppool,
        ):
            wt = cpool.tile([128, 128], mybir.dt.bfloat16)
            nc.sync.dma_start(out=wt[:], in_=wmat[:])

            # Issue every input-tile DMA upfront (sync queue blocks at
            # trigger k>=bufs until tile k-bufs is consumed, which is fine):
            # the read stream starts during the framework preamble and never
            # waits on the compute loop's program order.
            xts = []
            for i in range(N_TILES):
                xt = xpool.tile([128, F_TILE], mybir.dt.bfloat16)
                nc.sync.dma_start(out=xt[:], in_=xin[i])
                xts.append(xt)

            for i in range(N_TILES):
                xt = xts[i]
                yt = ypool.tile([128, F_TILE], mybir.dt.bfloat16)
                for m in range(F_TILE // 512):
                    f0 = m * 512
                    pt = ppool.tile([128, 512], mybir.dt.float32)
                    nc.tensor.matmul(
                        pt[:], lhsT=wt[:], rhs=xt[:, f0:f0 + 512],
                        start=True, stop=True)
                    if m % 2 == 0:
                        nc.scalar.copy(yt[:, f0:f0 + 512], pt[:])
                    else:
                        nc.vector.tensor_copy(yt[:, f0:f0 + 512], pt[:])

                nc.scalar.dma_start(out=yout[i], in_=yt[:])

    if legalize:
        _legalize_waits(nc)
    return nc


def _legalize_waits(nc, limit=1):
    """walrus codegen rejects instructions carrying more than ~1 sem wait
    (e.g. Matmult's LoadWeights slot).  Move excess waits onto NoOp
    instructions inserted just before the instruction on the same engine
    queue -- semantically identical (all waits still precede execution)."""
    import bass_rust

    fn = nc.m.functions[0]
    lastblk = fn.blocks[-1]
    eng_ns = {
        "PE": nc.tensor, "DVE": nc.vector, "Activation": nc.scalar,
        "SP": nc.sync, "Pool": nc.gpsimd,
    }
    # NoOp codegen requires >=1 sem update. Give each engine its own dummy
    # sem (ids picked from the top of the 150..255 HW range, skipping any id
    # already referenced) so no counting or cross-proc rule is disturbed.
    used_ids = set()
    for blk in fn.blocks:
        for inst in blk.instructions:
            si = getattr(inst, "sync_info", None)
            if si is None:
                continue
            for w in si.on_wait:
                used_ids.add(w.id)
            for upd in si.on_update:
                used_ids.add(upd.id)
    avail = [i for i in range(255, 149, -1) if i not in used_ids]
    eng_upd = {}
    for k, en in enumerate(["PE", "DVE", "Activation", "SP", "Pool"]):
        eng_upd[en] = bass_rust.SyncUpdate(
            sync_type="semaphore", id=avail[k], ant_name=f"waitnop_{en}",
            update_mode="sem-inc", update_value=1, update_reg=None)

    def copy_wait(w):
        return bass_rust.SyncWait(
            sync_type=w.sync_type, id=w.id, ant_name=w.ant_name,
            wait_mode=w.wait_mode, wait_value=w.wait_value, wait_reg=w.wait_reg)

    def make_nop(engine_name, waits):
        ns = eng_ns[engine_name]
        ns.nop(hint="waitcarrier")
        nop = lastblk.instructions.pop()
        raw = getattr(nop, "inst", nop)
        raw.sync_info = bass_rust.SyncInfo(
            on_wait=[copy_wait(w) for w in waits],
            on_update=[eng_upd[engine_name]])
        return raw

    for blk in fn.blocks:
        insts = blk.instructions
        i = 0
        while i < len(insts):
            inst = insts[i]
            ty = type(inst).__name__
            si = getattr(inst, "sync_info", None)
            if (ty not in ("InstEventSemaphore", "InstNoOp")
                    and si is not None and len(si.on_wait) > limit):
                ename = str(inst.engine).split(".")[-1]
                waits = [copy_wait(w) for w in si.on_wait]
                upds = list(si.on_update)
                extra, keep = waits[:-limit], waits[-limit:]
                for w in extra:
                    insts.insert(i, make_nop(ename, [w]))
                    i += 1
                inst.sync_info = bass_rust.SyncInfo(
                    on_wait=keep, on_update=upds)
            i += 1


def _make_wmat(w):
    """128x128 stationary butterfly: wm[p, q] with p = dt*64+dh*32+dw*16+g,
    q = k*16+g, value w[k,0,dt,dh,dw].  Fully general in w."""
    w = np.asarray(w, dtype=np.float32).reshape(8, 2, 2, 2)
    wm = np.zeros((128, 128), dtype=np.float32)
    g = np.arange(16)
    for k in range(8):
        for dt in range(2):
            for dh in range(2):
                for dw in range(2):
                    wm[dt * 64 + dh * 32 + dw * 16 + g, k * 16 + g] = \
                        w[k, dt, dh, dw]
    return wm


def _pack_input(x16):
    """(B,C,T_IN,512,512) bf16 -> (128, COLS_TOTAL) device column layout."""
    t = np.arange(T_OUT)
    t0 = np.maximum(2 * t - 1, 0)
    t1 = 2 * t
    fp = np.stack([x16[:, :, t0], x16[:, :, t1]], axis=3)  # b c t dt 512 512
    v = fp.reshape(N_UNITS, 2, 16, 16, 2, 256, 2)          # u dt g j dh wp dw
    v = v.transpose(0, 1, 4, 6, 2, 3, 5)                   # u dt dh dw g j wp
    p = v.reshape(N_UNITS, 128, UNIT_COLS)
    return p.transpose(1, 0, 2).reshape(128, COLS_TOTAL)


def _unpack_output(yg):
    """(128, COLS_TOTAL) bf16 device layout -> (2, 24, 17, 256, 256) f32."""
    q = yg.reshape(128, N_UNITS, UNIT_COLS).transpose(1, 0, 2)
    planes = q.reshape(N_UNITS, 8, HO, WO)                 # u k (g j)=ho wp
    out = planes.reshape(B, C, T_OUT, 8, HO, WO)
    return np.ascontiguousarray(
        out.transpose(0, 3, 1, 2, 4, 5)).reshape(
        B, 8 * C, T_OUT, HO, WO).astype(np.float32)


LAST_RESULT = None


def kernel(x, w):
    import os
    import ml_dtypes
    from concourse.bass_utils import run_bass_kernel_spmd

    bf16 = ml_dtypes.bfloat16
    x16 = np.asarray(x, dtype=np.float32).astype(bf16)
    wm = _make_wmat(w).astype(bf16)

    g = _pack_input(x16)
    in_maps = []
    for m in range(N_CORES):
        sl = g[:, m * COLS_PER_CORE:(m + 1) * COLS_PER_CORE]
        t3 = np.ascontiguousarray(
            sl.reshape(128, N_TILES, F_TILE).transpose(1, 0, 2))
        in_maps.append({"xin": t3, "wmat": wm})

    nc = _build_nc()
    kw = {}
    if os.environ.get("KERNEL_PROFILE") == "1":
        kw = dict(trace=True, tmpdir=os.environ.get("KERNEL_PROFILE_DIR"))
    res = run_bass_kernel_spmd(nc, in_maps, core_ids=list(range(N_CORES)), **kw)
    global LAST_RESULT
    LAST_RESULT = res

    yg = np.concatenate(
        [np.asarray(res.results[m]["yout"]).transpose(1, 0, 2).reshape(
            128, COLS_PER_CORE) for m in range(N_CORES)], axis=1)
    return _unpack_output(yg)


if __name__ == "__main__":
    x = np.random.randn(B, C, T_IN, H, W).astype(np.float32)
    SCALE = 0.3536
    flags = np.array([[0, 0, 0], [0, 0, 1], [0, 1, 0], [0, 1, 1],
                      [1, 0, 0], [1, 0, 1], [1, 1, 0], [1, 1, 1]])
    t, h, ww = np.meshgrid(np.arange(2), np.arange(2), np.arange(2), indexing="ij")
    sign = (-1.0) ** (flags[:, 0, None, None, None] * t
                      + flags[:, 1, None, None, None] * h
                      + flags[:, 2, None, None, None] * ww)
    wf = (SCALE * sign).reshape(8, 1, 2, 2, 2).astype(np.float32)
    y = kernel(x, wf)
    print(y.shape, y.dtype)
